# revision 43
# baseline (speedup 1.0000x reference)
"""Trainium2 Bass kernel for the DVNDTA GNN message-passing model.

Self-contained: host-side preprocessing (numpy) + Bass/Tile program builder +
axon-PJRT SPMD runner. Accepts FULL inputs, returns FULL output [256] f32.

Sharding: nodes are split into 8 contiguous ranges of 12500 (graph-sorted batch
means pooling stays mostly local; boundary graphs fixed by an AllReduce).
Each core owns the edges whose dst falls in its range. Per layer each core
computes hW1|hW2 for its own nodes, AllGathers the table, then gathers rows by
src (indirect DMA), builds scaled one-hot tiles ON DEVICE (one DVE
tensor_scalar per 128-edge tile: (iota==slot)*scale), and scatter-adds via PE
matmuls into [H x 128-node-window] PSUM accumulators. Degree normalization and
per-layer biases are folded into per-edge scales and a rank-1 (outer-product)
matmul per window. Pooling = PE transpose + one-hot matmul + 128KB AllReduce;
the FC head (BN affine folded into weights on host) runs redundantly on every
core; core 0's output is returned.

The wire format is minimized (host->device link is ~100 MB/s): bf16 node
features, packed per-edge {row id i32, slot bf16, scale bf16}, layer/head
weights sharded 1/8 per core and AllGathered on device.
"""
import sys
sys.path.insert(0, "/opt/trn_rl_repo")

import numpy as np
import ml_dtypes

import concourse.bass as bass
import concourse.mybir as mybir
from concourse.tile import TileContext

# ---------------------------------------------------------------- constants
NC = 8
N = 100000
G = 256
H = 128
N_LAYERS = 4
BN_EPS = 1e-5
NPC = N // NC            # 12500 real nodes per core
NP = 12544               # padded (98 windows of 128)
NW = NP // 128           # 98
TBL_ROWS = NC * 2 * NP   # 200704
F32 = mybir.dt.float32
I32 = mybir.dt.int32
U16 = mybir.dt.uint16
U8 = mybir.dt.uint8
BF16 = mybir.dt.bfloat16
FP8 = mybir.dt.float8e4
BF16_NP = ml_dtypes.bfloat16

_cache = {}


def _layout(B):
    """Byte layout of the per-core u8 input blob: name -> (offset, nbytes,
    mybir dtype, partition count, free count). All sizes are multiples of 4,
    so every field stays 4-byte aligned regardless of order."""
    fields = [
        ("ed_u16", U16, 2, 128, B),
        ("ed_u8", U8, 1, 128, B),
        ("ed_fp8", FP8, 1, 128, B),
        ("nod", FP8, 1, 35, NP),
        ("lin_W", BF16, 2, 35, 128),
        ("bat", BF16, 2, 128, NW),
        ("Wsh", BF16, 2, 64, 256),
        ("Hsh", BF16, 2, 48, 128),
        ("f_i", BF16, 2, 1, NP),
        ("f_e", BF16, 2, 1, NP),
        ("lin_b", F32, 4, 128, 1),
        ("bcat", BF16, 2, 1, 2 * N_LAYERS * 128),
        ("headb", F32, 4, 1, 3 * 128),
        ("outW", F32, 4, 128, 1),
        ("outb", F32, 4, 1, 1),
    ]
    lay, off = {}, 0
    for name, dtt, esz, p, f in fields:
        nb = esz * p * f
        lay[name] = (off, nb, dtt, p, f)
        off += nb
    return lay, off


# ---------------------------------------------------------------- wait fix
def _legalize_waits(nc, max_waits=1):
    """This container's walrus rejects >1 sync-wait per instruction; hoist
    extras onto standalone same-engine NoOps (the raw-bass wait_ge shape)."""
    n = 0
    for fn in nc.m.functions:
        for blk in fn.blocks:
            new_insts = []
            for inst in blk.instructions:
                si = inst.sync_info
                waits = list(si.on_wait) if si is not None and si.on_wait else []
                if len(waits) > max_waits:
                    for w in waits[:-1]:
                        nop = mybir.InstNoOp(
                            name=f"{inst.name}-wf-{n}", engine=inst.engine,
                            ins=[], outs=[],
                            sync_info=mybir.SyncInfo(on_wait=[w], on_update=[]))
                        new_insts.append(nop)
                        n += 1
                    inst.sync_info = mybir.SyncInfo(
                        on_wait=[waits[-1]], on_update=list(si.on_update or []))
                new_insts.append(inst)
            blk.instructions.clear()
            for i in new_insts:
                blk.instructions.append(i)
    return n


# ---------------------------------------------------------------- host prep
def _edge_streams(rows, dst, scale):
    """Per-core, per-window padded edge streams (fully vectorized).

    `rows` is the 17-bit (rank<<14 | q) id; packed 24-bit v = rows<<7 | slot
    is shipped as hi-u16 (v>>8) + lo-u8 (v&255), 3 bytes/lane.
    Returns (tiles_per_window [NW] shared across cores, T,
             HI [NC,128,T] u16, LO [NC,128,T] u8, V [NC,128,T] fp8 scales).
    Pad lanes: packed 0 (row 0 of rank 0), scale 0.
    """
    dst = dst.astype(np.int32)
    core = dst // NPC
    dl = dst - core * NPC
    win = dl >> 7
    slot = dl & 127
    wg = (core * NW + win).astype(np.int16)  # < 784; int16 => radix argsort
    counts = np.bincount(wg, minlength=NC * NW)
    cmax = counts.reshape(NC, NW).max(axis=0)
    tpw = np.maximum(1, -(-cmax // 128))
    T = int(tpw.sum())
    lane0 = np.zeros(NW, np.int64)
    lane0[1:] = np.cumsum(tpw)[:-1]
    lane0 *= 128
    order = np.argsort(wg, kind="stable")
    wg_s = wg[order]
    csum = np.zeros(NC * NW + 1, np.int64)
    csum[1:] = np.cumsum(counts)
    lane = (np.arange(len(wg_s)) - csum[wg_s]) + lane0[wg_s % NW]
    core_s = wg_s // NW
    P = np.zeros((NC, T * 128), np.int32)
    V = np.zeros((NC, T * 128), np.float32)
    P[core_s, lane] = (rows[order].astype(np.int32) << 7) | slot[order].astype(np.int32)
    V[core_s, lane] = scale[order]

    def tp(a):  # [NC, T*128] -> [NC, 128, T] (lane = t*128 + p)
        return np.ascontiguousarray(a.reshape(NC, T, 128).transpose(0, 2, 1))
    return (tpw.astype(int).tolist(), T, tp((P >> 8).astype(np.uint16)),
            tp((P & 255).astype(np.uint8)), tp(V.astype(ml_dtypes.float8_e4m3fn)))


def _preprocess(x, edge_index_intra, edge_index_inter, pos, edge_attr, batch,
                lin_node_W, lin_node_b, W_intra, b_intra, W_inter, b_inter,
                fc_W, fc_b, bn_gamma, bn_beta, out_W, out_b):
    x = np.asarray(x); pos = np.asarray(pos); batch = np.asarray(batch)
    ei = np.asarray(edge_index_intra); ee = np.asarray(edge_index_inter)
    ea = np.asarray(edge_attr).reshape(-1)

    src_i, dst_i = ei[0].astype(np.int64), ei[1].astype(np.int64)
    src_e, dst_e = ee[0].astype(np.int64), ee[1].astype(np.int64)

    deg_i = np.bincount(dst_i, minlength=N).astype(np.float32)
    cnt_e = np.bincount(dst_e, minlength=N).astype(np.float32)
    logdeg_e = np.log(cnt_e + 1.0)

    # per-edge scales with degree norm folded in
    scale_i = (ea / (deg_i[dst_i] + 1.0)).astype(np.float32)
    d2 = ((pos[src_e] - pos[dst_e]) ** 2).sum(axis=1)
    scale_e = (np.exp(-d2) * logdeg_e[dst_e]).astype(np.float32)

    # 17-bit src id: rank<<14 | q.  The device expands to the table row
    # r*2*NP + q, adding +NP for inter columns.
    def rqid(s):
        s = s.astype(np.int32)
        r = s // NPC
        return (r << 14) | (s - r * NPC)

    tpw_i, TI, HI_i, LO_i, V_i = _edge_streams(rqid(src_i), dst_i, scale_i)
    tpw_e, TE, HI_e, LO_e, V_e = _edge_streams(rqid(src_e), dst_e, scale_e)
    B = TI + TE

    # rank-1 bias factors per node
    f_i = deg_i / (deg_i + 1.0)
    f_e = cnt_e * logdeg_e

    # head: fold BN affine into the next layer's weights
    s = np.float32(1.0 / np.sqrt(1.0 + BN_EPS))
    A = [s * np.asarray(bn_gamma)[j] for j in range(3)]
    Bt = [np.asarray(bn_beta)[j] for j in range(3)]
    fcW = [np.asarray(fc_W)[j] for j in range(3)]
    fcb = [np.asarray(fc_b)[j] for j in range(3)]
    hW = [fcW[0], A[0][:, None] * fcW[1], A[1][:, None] * fcW[2]]
    hb = [fcb[0], fcb[1] + Bt[0] @ fcW[1], fcb[2] + Bt[1] @ fcW[2]]
    oW = (A[2][:, None] * np.asarray(out_W)).astype(np.float32)      # [H,1]
    ob = np.float32(np.asarray(out_b)[0] + Bt[2] @ np.asarray(out_W))

    # sharded big params: Wcat [4*128, 256] and headW [3*128, 128]
    Wcat = np.concatenate([
        np.concatenate([np.asarray(W_intra)[l], np.asarray(W_inter)[l]], axis=1)
        for l in range(N_LAYERS)], axis=0).astype(np.float32)        # [512,256]
    headW = np.concatenate(hW, axis=0).astype(np.float32)            # [384,128]
    bcat = np.concatenate([np.asarray(b_intra), np.asarray(b_inter)])  # [8,128]

    FP8_NP = ml_dtypes.float8_e4m3fn
    x_q = x.astype(FP8_NP)
    linW_bf = np.asarray(lin_node_W).astype(BF16_NP)

    lay, TOT = _layout(B)
    in_maps = []
    for c in range(NC):
        lo, hi = c * NPC, (c + 1) * NPC
        nod = np.zeros((35, NP), FP8_NP)
        nod[:, :NPC] = x_q[lo:hi].T
        bl = np.full(NP, -5.0, np.float32)
        bl[:NPC] = batch[lo:hi].astype(np.float32)
        fi = np.zeros(NP, np.float32); fi[:NPC] = f_i[lo:hi]
        fe = np.zeros(NP, np.float32); fe[:NPC] = f_e[lo:hi]
        arrs = {
            "ed_u16": np.concatenate([HI_i[c], HI_e[c]], axis=1),    # [128, B]
            "ed_u8": np.concatenate([LO_i[c], LO_e[c]], axis=1),     # [128, B]
            "ed_fp8": np.concatenate([V_i[c], V_e[c]], axis=1),      # [128, B]
            "nod": nod,
            "lin_W": linW_bf,
            "bat": bl.reshape(NW, 128).T.astype(BF16_NP),            # [128, NW]
            "Wsh": Wcat[c * 64:(c + 1) * 64].astype(BF16_NP),        # [64, 256]
            "Hsh": headW[c * 48:(c + 1) * 48].astype(BF16_NP),       # [48,128]
            "f_i": fi[None, :].astype(BF16_NP),
            "f_e": fe[None, :].astype(BF16_NP),
            "lin_b": np.asarray(lin_node_b).astype(np.float32)[:, None],
            "bcat": bcat.astype(BF16_NP),
            "headb": np.stack(hb).astype(np.float32),
            "outW": oW.reshape(H, 1),
            "outb": np.full((1, 1), ob, np.float32),
        }
        blob = np.empty((1, TOT), np.uint8)
        for name, (off, nb, _d, _p, _f) in lay.items():
            blob[0, off:off + nb] = np.ascontiguousarray(
                arrs[name]).view(np.uint8).reshape(-1)
        in_maps.append({"blob": blob})
    return tpw_i, TI, tpw_e, TE, in_maps


# ---------------------------------------------------------------- program
def _build(tpw_i, TI, tpw_e, TE, ablate=(), n_layers=N_LAYERS):
    B = TI + TE

    lay, TOT = _layout(B)
    nc = bass.Bass()
    blob = nc.declare_dram_parameter("blob", [1, TOT], U8, isOutput=False)
    out = nc.declare_dram_parameter("out", [1, G], F32, isOutput=True)

    def bview(name):
        off, nb, dtt, p, f = lay[name]
        v = blob[0:1, off:off + nb].bitcast(dtt)
        if p > 1:
            v = v.rearrange("o (p t) -> (o p) t", p=p)
        return v

    ed_u16, ed_u8, ed_fp8 = bview("ed_u16"), bview("ed_u8"), bview("ed_fp8")
    nod, lin_W, bat = bview("nod"), bview("lin_W"), bview("bat")
    Wsh, Hsh = bview("Wsh"), bview("Hsh")
    f_i, f_e, lin_b = bview("f_i"), bview("f_e"), bview("lin_b")
    bcat, outW, outb = bview("bcat"), bview("outW"), bview("outb")
    ho, hn = lay["headb"][0], lay["headb"][1]
    headb = blob[0:1, ho:ho + hn].bitcast(F32).rearrange(
        "o (j h) -> (o h) j", h=128)

    Wloc = nc.dram_tensor("Wloc", [64, 256], BF16)
    Hloc = nc.dram_tensor("Hloc", [48, 128], BF16)
    Wfull = nc.dram_tensor("Wfull", [512, 256], BF16, addr_space="Shared")
    Hfull = nc.dram_tensor("Hfull", [384, 128], BF16, addr_space="Shared")
    hWcat = nc.dram_tensor("hWcat", [2 * NP, 128], BF16)
    table = nc.dram_tensor("table", [TBL_ROWS, 128], BF16, addr_space="Shared")
    g_loc = nc.dram_tensor("g_loc", [128, G], F32)
    g_sh = nc.dram_tensor("g_sh", [128, G], F32, addr_space="Shared")

    Silu = mybir.ActivationFunctionType.Silu
    Lrelu = mybir.ActivationFunctionType.Lrelu
    AG = mybir.AluOpType

    with TileContext(nc) as tc:
        with (
            tc.tile_pool(name="persist", bufs=1) as pp,
            tc.tile_pool(name="gath", bufs=8) as gp,
            tc.tile_pool(name="sel", bufs=4) as sp,
            tc.tile_pool(name="upd", bufs=2) as up,
            tc.tile_pool(name="ps", bufs=2, space="PSUM") as ps,
            tc.tile_pool(name="pedge", bufs=2, space="PSUM") as pe,
        ):
            # ---- param AllGathers (sharded upload of Wcat / headW);
            # collectives cannot read IO tensors, so stage via internal DRAM
            nc.gpsimd.dma_start(out=Wloc[:], in_=Wsh)
            nc.gpsimd.dma_start(out=Hloc[:], in_=Hsh)
            if "nocc" not in ablate:
                nc.gpsimd.collective_compute(
                    "AllGather", AG.bypass, ins=[Wloc[:]], outs=[Wfull[:]],
                    replica_groups=[list(range(NC))])
                nc.gpsimd.collective_compute(
                    "AllGather", AG.bypass, ins=[Hloc[:]], outs=[Hfull[:]],
                    replica_groups=[list(range(NC))])
            else:
                nc.gpsimd.dma_start(out=Wfull[0:64, :], in_=Wloc[:])
                nc.gpsimd.dma_start(out=Hfull[0:48, :], in_=Hloc[:])
            dummy0 = up.tile([1, 128], F32, tag="dummy")
            nc.gpsimd.dma_start(out=dummy0[:], in_=Wfull[0:1, 0:128])

            # ---- constants / streams (SBUF-resident)
            iota_i = pp.tile([128, 128], I32)
            nc.gpsimd.iota(iota_i[:], pattern=[[1, 128]], base=0, channel_multiplier=0)
            iota_f = pp.tile([128, 128], F32)
            nc.vector.tensor_copy(out=iota_f[:], in_=iota_i[:])

            t_linW = pp.tile([35, 128], BF16)
            nc.sync.dma_start(out=t_linW[:], in_=lin_W)
            t_linb = pp.tile([128, 1], F32)
            nc.sync.dma_start(out=t_linb[:], in_=lin_b)
            t_Wcat = pp.tile([128, N_LAYERS * 256], F32)
            with tc.tile_pool(name="wcv", bufs=2) as wcv:
                for l in range(N_LAYERS):
                    wtmp = wcv.tile([128, 256], BF16, tag="w")
                    nc.sync.dma_start(out=wtmp[:],
                                      in_=Wfull[l * 128:(l + 1) * 128, :])
                    nc.vector.tensor_copy(out=t_Wcat[:, l * 256:(l + 1) * 256],
                                          in_=wtmp[:])
            # biases on partition 0 (matmul lhsT base partition must be 0/32/64)
            t_bcat = pp.tile([1, 2 * N_LAYERS * 128], BF16)
            nc.sync.dma_start(out=t_bcat[:],
                              in_=bcat)
            # unpack 24-bit (rank<<21 | q<<7 | slot) from u16 hi + u8 lo ->
            # rows i32 (r*2NP + q, +NP for inter cols), slot f32; scales fp8->f32
            t_rows = pp.tile([128, B], I32)
            t_ss = pp.tile([128, 2 * B], F32)
            CV = 256
            with tc.tile_pool(name="cvt", bufs=1) as cvt:
                for c0 in range(0, B, CV):
                    cw = min(CV, B - c0)
                    hi8 = cvt.tile([128, CV], U16, tag="hi8")
                    nc.sync.dma_start(out=hi8[:, :cw], in_=ed_u16[:, c0:c0 + cw])
                    lo8 = cvt.tile([128, CV], U8, tag="lo8")
                    nc.sync.dma_start(out=lo8[:, :cw], in_=ed_u8[:, c0:c0 + cw])
                    A = cvt.tile([128, CV], I32, tag="A")
                    L = cvt.tile([128, CV], I32, tag="L")
                    Q = cvt.tile([128, CV], I32, tag="Q")
                    S = cvt.tile([128, CV], I32, tag="S")
                    nc.vector.tensor_copy(out=A[:, :cw], in_=hi8[:, :cw])
                    nc.vector.tensor_copy(out=L[:, :cw], in_=lo8[:, :cw])
                    nc.vector.tensor_scalar(out=A[:, :cw], in0=A[:, :cw], scalar1=8,
                                            scalar2=None, op0=AG.logical_shift_left)
                    nc.vector.tensor_tensor(out=A[:, :cw], in0=A[:, :cw],
                                            in1=L[:, :cw], op=AG.bitwise_or)
                    nc.vector.tensor_scalar(out=S[:, :cw], in0=A[:, :cw], scalar1=127,
                                            scalar2=None, op0=AG.bitwise_and)
                    nc.vector.tensor_copy(out=t_ss[:, c0:c0 + cw], in_=S[:, :cw])
                    nc.vector.tensor_scalar(out=L[:, :cw], in0=A[:, :cw], scalar1=7,
                                            scalar2=None, op0=AG.arith_shift_right)
                    nc.vector.tensor_scalar(out=Q[:, :cw], in0=L[:, :cw],
                                            scalar1=16383, scalar2=None,
                                            op0=AG.bitwise_and)
                    nc.vector.tensor_scalar(out=L[:, :cw], in0=L[:, :cw], scalar1=14,
                                            scalar2=None, op0=AG.arith_shift_right)
                    nc.vector.tensor_scalar(out=L[:, :cw], in0=L[:, :cw],
                                            scalar1=2 * NP, scalar2=None,
                                            op0=AG.mult)
                    nc.vector.tensor_tensor(out=t_rows[:, c0:c0 + cw],
                                            in0=L[:, :cw], in1=Q[:, :cw], op=AG.add)
                    # inter columns gather from the +NP half of each rank block
                    a = max(TI, c0)
                    if a < c0 + cw:
                        nc.vector.tensor_scalar(
                            out=t_rows[:, a:c0 + cw], in0=t_rows[:, a:c0 + cw],
                            scalar1=NP, scalar2=None, op0=AG.add)
                    sc = cvt.tile([128, CV], FP8, tag="sc")
                    nc.sync.dma_start(out=sc[:, :cw], in_=ed_fp8[:, c0:c0 + cw])
                    nc.vector.tensor_copy(out=t_ss[:, B + c0:B + c0 + cw],
                                          in_=sc[:, :cw])
            vp_t = pp.tile([128, NP], F32)
            vl_t = pp.tile([128, NP], F32)
            t_blo = pp.tile([128, NW], F32)
            t_bhi = pp.tile([128, NW], F32)
            with tc.tile_pool(name="cvb", bufs=1) as cvb:
                tmpb = cvb.tile([128, NW], BF16)
                nc.sync.dma_start(out=tmpb[:], in_=bat)
                nc.vector.tensor_copy(out=t_blo[:], in_=tmpb[:])
            nc.vector.tensor_scalar(out=t_bhi[:], in0=t_blo[:], scalar1=128.0,
                                    scalar2=None, op0=AG.subtract)

            # ---- h0 = silu(x @ lin_W + b), H-major [128, NP]
            h = pp.tile([128, NP], F32)
            CH = 512
            with tc.tile_pool(name="xtp", bufs=3) as xtp:
                for i in range(0, NP, CH):
                    w = min(CH, NP - i)
                    x8 = xtp.tile([35, CH], FP8, tag="x8")
                    nc.sync.dma_start(out=x8[:, :w], in_=nod[:, i:i + w])
                    xc = xtp.tile([35, CH], BF16, tag="xc")
                    nc.vector.tensor_copy(out=xc[:, :w], in_=x8[:, :w])
                    p_h0 = ps.tile([128, CH], F32, tag="ps")
                    nc.tensor.matmul(out=p_h0[:, :w], lhsT=t_linW[:],
                                     rhs=xc[:, :w], start=True, stop=True)
                    nc.scalar.activation(out=h[:, i:i + w], in_=p_h0[:, :w],
                                         func=Silu, bias=t_linb[:])

            start_i = np.concatenate([[0], np.cumsum(tpw_i)]).astype(int)
            start_e = np.concatenate([[0], np.cumsum(tpw_e)]).astype(int)

            # ---- layers
            for ll in range(n_layers):
                l = ll % N_LAYERS
                # hW1|hW2 for own nodes -> hWcat -> AllGather -> table
                for w in range(NW):
                    p_hw = ps.tile([128, 256], F32, tag="ps")
                    nc.tensor.matmul(out=p_hw[:], lhsT=h[:, w * 128:(w + 1) * 128],
                                     rhs=t_Wcat[:, l * 256:(l + 1) * 256],
                                     start=True, stop=True)
                    stg = sp.tile([128, 256], BF16, tag="stg")
                    nc.scalar.copy(out=stg[:], in_=p_hw[:])
                    nc.sync.dma_start(out=hWcat[w * 128:(w + 1) * 128, :],
                                      in_=stg[:, 0:128])
                    nc.sync.dma_start(out=hWcat[NP + w * 128:NP + (w + 1) * 128, :],
                                      in_=stg[:, 128:256])
                if "nocc" not in ablate:
                    nc.gpsimd.collective_compute(
                        "AllGather", AG.bypass, ins=[hWcat[:]], outs=[table[:]],
                        replica_groups=[list(range(NC))])
                else:
                    nc.gpsimd.dma_start(out=table[0:2 * NP, :], in_=hWcat[:])
                # funnel: absorb the collective wait on the gpsimd queue
                dummy = up.tile([1, 128], F32, tag="dummy")
                nc.gpsimd.dma_start(out=dummy[:], in_=table[0:1, :])

                for w in range(NW):
                    wsl = slice(w * 128, (w + 1) * 128)
                    p_mi = pe.tile([128, 128], F32, tag="mi")
                    p_me = pe.tile([128, 128], F32, tag="me")
                    # rank-1 bias terms init the accumulators
                    t_fiw = sp.tile([1, 128], BF16, tag="fiw")
                    t_few = sp.tile([1, 128], BF16, tag="few")
                    nc.sync.dma_start(out=t_fiw[:], in_=f_i[:, wsl])
                    nc.sync.dma_start(out=t_few[:], in_=f_e[:, wsl])
                    nc.tensor.matmul(out=p_mi[:], lhsT=t_bcat[:, l * 128:(l + 1) * 128],
                                     rhs=t_fiw[:], start=True,
                                     stop=(tpw_i[w] == 0), skip_group_check=True)
                    nc.tensor.matmul(out=p_me[:], lhsT=t_bcat[:, (N_LAYERS + l) * 128:(N_LAYERS + l + 1) * 128],
                                     rhs=t_few[:], start=True,
                                     stop=(tpw_e[w] == 0), skip_group_check=True)
                    for (s0, s1, tile_off, p_acc) in (
                        (start_i[w], start_i[w + 1], 0, p_mi),
                        (start_e[w], start_e[w + 1], TI, p_me),
                    ):
                        for t in range(s0, s1):
                            if "edges" in ablate:
                                break
                            tg = tile_off + t
                            st = sp.tile([128, 128], BF16, tag="st")
                            nc.vector.tensor_scalar(
                                out=st[:], in0=iota_f[:],
                                scalar1=t_ss[:, tg:tg + 1],
                                scalar2=t_ss[:, B + tg:B + tg + 1],
                                op0=AG.is_equal, op1=AG.mult)
                            gt = gp.tile([128, 128], BF16, tag="gt")
                            if "gather" not in ablate:
                                nc.gpsimd.indirect_dma_start(
                                    out=gt[:], out_offset=None, in_=table[:],
                                    in_offset=bass.IndirectOffsetOnAxis(
                                        ap=t_rows[:, tg:tg + 1], axis=0))
                            if "mm" not in ablate:
                                nc.tensor.matmul(out=p_acc[:], lhsT=gt[:],
                                                 rhs=st[:],
                                                 start=False, stop=(t == s1 - 1),
                                                 skip_group_check=True)
                    # update: vp = silu(m_i + vp); vl = silu(m_e + vl); h += vp+vl
                    if l == 0:
                        nc.scalar.activation(out=vp_t[:, wsl], in_=p_mi[:], func=Silu)
                        nc.scalar.activation(out=vl_t[:, wsl], in_=p_me[:], func=Silu)
                    else:
                        t1 = up.tile([128, 128], F32, tag="t1")
                        t2 = up.tile([128, 128], F32, tag="t2")
                        nc.vector.tensor_tensor(out=t1[:], in0=p_mi[:], in1=vp_t[:, wsl], op=AG.add)
                        nc.vector.tensor_tensor(out=t2[:], in0=p_me[:], in1=vl_t[:, wsl], op=AG.add)
                        nc.scalar.activation(out=vp_t[:, wsl], in_=t1[:], func=Silu)
                        nc.scalar.activation(out=vl_t[:, wsl], in_=t2[:], func=Silu)
                    nc.vector.tensor_tensor(out=h[:, wsl], in0=h[:, wsl], in1=vp_t[:, wsl], op=AG.add)
                    nc.vector.tensor_tensor(out=h[:, wsl], in0=h[:, wsl], in1=vl_t[:, wsl], op=AG.add)

            # ---- global_add_pool: gT[H, 256] via transpose + one-hot matmuls
            do_pool = "nopool" not in ablate
            from concourse.masks import make_identity
            ident = pp.tile([128, 128], F32)
            make_identity(nc, ident[:])
            p_glo = pe.tile([128, 128], F32, tag="mi")
            p_ghi = pe.tile([128, 128], F32, tag="mi")
            for w in range(NW if do_pool else 1):
                wsl = slice(w * 128, (w + 1) * 128)
                p_t = pe.tile([128, 128], F32, tag="me")
                nc.tensor.transpose(out=p_t[:], in_=h[:, wsl], identity=ident[:])
                X = sp.tile([128, 128], F32, tag="X")
                nc.scalar.copy(out=X[:], in_=p_t[:])
                Slo = sp.tile([128, 128], F32, tag="Slo")
                Shi = sp.tile([128, 128], F32, tag="Shi")
                nc.vector.tensor_scalar(out=Slo[:], in0=iota_f[:],
                                        scalar1=t_blo[:, w:w + 1], scalar2=None,
                                        op0=AG.is_equal)
                nc.vector.tensor_scalar(out=Shi[:], in0=iota_f[:],
                                        scalar1=t_bhi[:, w:w + 1], scalar2=None,
                                        op0=AG.is_equal)
                last = (w == (NW - 1 if do_pool else 0))
                nc.tensor.matmul(out=p_glo[:], lhsT=X[:], rhs=Slo[:],
                                 start=(w == 0), stop=last,
                                 skip_group_check=True)
                nc.tensor.matmul(out=p_ghi[:], lhsT=X[:], rhs=Shi[:],
                                 start=(w == 0), stop=last,
                                 skip_group_check=True)
            gsb = up.tile([128, G], F32, tag="gsb")
            nc.vector.tensor_copy(out=gsb[:, 0:128], in_=p_glo[:])
            nc.vector.tensor_copy(out=gsb[:, 128:256], in_=p_ghi[:])
            nc.sync.dma_start(out=g_loc[:], in_=gsb[:])
            if "nocc" not in ablate:
                nc.gpsimd.collective_compute(
                    "AllReduce", AG.add, ins=[g_loc[:]], outs=[g_sh[:]],
                    replica_groups=[list(range(NC))])
            else:
                nc.gpsimd.dma_start(out=g_sh[:], in_=g_loc[:])
            dummy2 = up.tile([1, 128], F32, tag="dummy")
            nc.gpsimd.dma_start(out=dummy2[:], in_=g_sh[0:1, 0:128])

            # ---- FC head (BN folded); gT layout [H, 256]
            t_hW = pp.tile([128, 3 * 128], F32)
            with tc.tile_pool(name="hcv", bufs=2) as hcv:
                for j in range(3):
                    htmp = hcv.tile([128, 128], BF16, tag="h")
                    nc.sync.dma_start(out=htmp[:],
                                      in_=Hfull[j * 128:(j + 1) * 128, :])
                    nc.vector.tensor_copy(out=t_hW[:, j * 128:(j + 1) * 128],
                                          in_=htmp[:])
            t_hb = pp.tile([128, 3], F32)
            nc.sync.dma_start(out=t_hb[:], in_=headb)
            t_oW = pp.tile([128, 1], F32)
            nc.sync.dma_start(out=t_oW[:], in_=outW)
            t_ob = pp.tile([1, 1], F32)
            nc.sync.dma_start(out=t_ob[:], in_=outb)

            gcur = up.tile([128, G], F32, tag="gcur")
            nc.sync.dma_start(out=gcur[:], in_=g_sh[:])
            for j in range(3):
                p_hd = ps.tile([128, G], F32, tag="ps")
                nc.tensor.matmul(out=p_hd[:], lhsT=t_hW[:, j * 128:(j + 1) * 128],
                                 rhs=gcur[:], start=True, stop=True)
                gnew = up.tile([128, G], F32, tag="gcur")
                nc.scalar.activation(out=gnew[:], in_=p_hd[:], func=Lrelu,
                                     bias=t_hb[:, j:j + 1], alpha=0.01)
                gcur = gnew
            p_o = ps.tile([1, G], F32, tag="ps")
            nc.tensor.matmul(out=p_o[:], lhsT=t_oW[:], rhs=gcur[:],
                             start=True, stop=True)
            osb = up.tile([1, G], F32, tag="osb")
            nc.vector.tensor_scalar(out=osb[:], in0=p_o[:],
                                    scalar1=t_ob[0:1, 0:1], scalar2=None,
                                    op0=AG.add)
            nc.sync.dma_start(out=out[:], in_=osb[:])

    _legalize_waits(nc)
    return nc


# ---------------------------------------------------------------- runner
class _Runner:
    def __init__(self, nc, n_cores=NC):
        import jax
        import hashlib
        from jax.sharding import Mesh, PartitionSpec
        from jax.experimental.shard_map import shard_map
        from concourse.bass2jax import (
            _bass_exec_p, install_neuronx_cc_hook, partition_id_tensor)
        install_neuronx_cc_hook()
        self.jax = jax
        self.n_cores = n_cores
        h = int.from_bytes(hashlib.sha256(nc.to_json_bytes()).digest()[:4], "little")
        self._cb_shape = [1, 1 + (h % 8191)]
        nc.declare_dram_parameter("zz_cachebust", self._cb_shape, I32, isOutput=False)

        partition_name = nc.partition_id_tensor.name if nc.partition_id_tensor else None
        in_names, out_names, out_avals, zero_outs = [], [], [], []
        for alloc in nc.m.functions[0].allocations:
            if not isinstance(alloc, mybir.MemoryLocationSet):
                continue
            name = alloc.memorylocations[0].name
            if alloc.kind == "ExternalInput":
                if name != partition_name:
                    in_names.append(name)
            elif alloc.kind == "ExternalOutput":
                shape = list(alloc.tensor_shape)
                dt = mybir.dt.np(alloc.dtype)
                out_names.append(name)
                out_avals.append(jax.core.ShapedArray(shape, dt))
                zero_outs.append(np.zeros(shape, dt))
        self.in_names, self.out_names = in_names, out_names
        self.out_avals, self.zero_outs = out_avals, zero_outs
        n_params, n_outs = len(in_names), len(out_avals)
        all_in = in_names + out_names + ([partition_name] if partition_name else [])

        def _body(*args):
            operands = list(args)
            if partition_name is not None:
                operands.append(partition_id_tensor())
            return tuple(_bass_exec_p.bind(
                *operands, out_avals=tuple(out_avals), in_names=tuple(all_in),
                out_names=tuple(out_names), lowering_input_output_aliases=(),
                sim_require_finite=False, sim_require_nnan=False, nc=nc))

        devices = jax.devices()[:n_cores]
        mesh = Mesh(np.asarray(devices), ("core",))
        self.fn = jax.jit(
            shard_map(_body, mesh=mesh,
                      in_specs=(PartitionSpec("core"),) * (n_params + n_outs),
                      out_specs=(PartitionSpec("core"),) * len(out_names),
                      check_rep=False),
            keep_unused=True)
        self.n_params = n_params

    def run(self, in_maps):
        jax = self.jax
        cb = np.zeros(self._cb_shape, np.int32)
        in_maps = [{**m, "zz_cachebust": cb} for m in in_maps]
        if not hasattr(self, "_bufs"):
            self._bufs = []
            for n in self.in_names:
                a0 = np.asarray(in_maps[0][n])
                self._bufs.append(np.empty(
                    (self.n_cores * a0.shape[0], *a0.shape[1:]), a0.dtype))
            self._zbufs = [np.zeros((self.n_cores * z.shape[0], *z.shape[1:]),
                                    z.dtype) for z in self.zero_outs]
        for i, n in enumerate(self.in_names):
            buf = self._bufs[i]
            r = buf.shape[0] // self.n_cores
            for c in range(self.n_cores):
                np.copyto(buf[c * r:(c + 1) * r], np.asarray(in_maps[c][n]))
        out_arrs = self.fn(*self._bufs, *self._zbufs)
        jax.block_until_ready(out_arrs)
        # every core computes the identical full output (the FC head runs
        # redundantly); fetch only core 0's shard — each shard fetch is a
        # blocking roundtrip over the axon tunnel (~10 ms apiece).
        res0 = {}
        for i, n in enumerate(self.out_names):
            s0 = min(out_arrs[i].addressable_shards,
                     key=lambda s: (s.index[0].start or 0))
            res0[n] = np.asarray(s0.data).reshape(self.out_avals[i].shape)
        return [res0] + [None] * (self.n_cores - 1)


# ---------------------------------------------------------------- entry
def kernel(**inputs):
    tpw_i, TI, tpw_e, TE, in_maps = _preprocess(**inputs)
    key = (TI, TE, tuple(tpw_i), tuple(tpw_e))
    if key not in _cache:
        nc = _build(tpw_i, TI, tpw_e, TE)
        _cache[key] = _Runner(nc)
    runner = _cache[key]
    res = runner.run(in_maps)
    return res[0]["out"].reshape(G).astype(np.float32)
